# revision 1
# baseline (speedup 1.0000x reference)
"""LightGCN-style GNN (3 mean-agg layers + review conv + edge-softmax attention)
on 8 Trainium2 NeuronCores.

Strategy: shard every phase by destination rows (8 contiguous ranges).  Each
core gathers source rows with int16-chunked `dma_gather`, reduces segments with
one-hot matmuls accumulated in PSUM (128-dst subwindows), normalizes with
host-precomputed inverse counts, and writes its shard.  Full tables needed by
the next phase are rebuilt with AllGather collectives.  All index manipulation
(sorting edges into (superwindow, chunk, subwindow) segments, int16 packing,
degree counts) happens on the host; all FLOPs and feature movement happen on
device.
"""

import os
import sys
import types

import numpy as np

# ---------------------------------------------------------------------------
# configuration (overridable for scaled-down testing)
# ---------------------------------------------------------------------------
CFG = {
    "R": 400_000,      # review nodes
    "M": 100_000,      # final dst nodes
    "L": 3,            # propagation layers
    "NCORE": 8,
    "CH": 32768,       # int16 gather chunk
    "SUB": 128,        # dst rows per subwindow (PSUM partitions)
    "NSUP": 16,        # subwindows per superwindow (e1/e2)
    "NSUP3": 8,        # subwindows per superwindow (e3; wider PSUM slots)
    "OHG": 8,          # one-hot build group (blocks per DVE op)
    "NQ": 4,           # SWDGE queues
    "TRACE": False,
}

_LAST = {"exec_ns": None, "profile_json": None}


def _install_profile_hook():
    try:
        if "antenv.axon_hooks" in sys.modules:
            return
        import antenv

        mod = types.ModuleType("antenv.axon_hooks")
        mod._hook = None
        mod.set_axon_ntff_profile_hook = lambda h: setattr(mod, "_hook", h)
        mod.get_axon_ntff_profile_hook = lambda: mod._hook
        sys.modules["antenv.axon_hooks"] = mod
        antenv.axon_hooks = mod
        from trn_agent_boot.trn_boot import _ntff_profile_via_ctypes

        mod.set_axon_ntff_profile_hook(
            _ntff_profile_via_ctypes("/opt/axon/libaxon_pjrt.so")
        )
    except Exception:
        pass


# ---------------------------------------------------------------------------
# host-side index preparation
# ---------------------------------------------------------------------------
class PhaseMeta:
    """Static (core-independent) structure of one gather/reduce phase."""

    def __init__(self, nsub, nsup, nchunk, table_rows, caps):
        self.nsub = nsub            # total subwindows (padded to nsup multiple)
        self.nsup = nsup
        self.nchunk = nchunk
        self.table_rows = table_rows
        self.caps = caps            # [nsub, nchunk] slot capacity (mult of 128)
        self.nsuper = nsub // nsup
        # piece (s, c) capacities & segment offsets
        self.seg_off = np.zeros((nsub, nchunk), np.int64)  # piece-local slot off
        self.piece_cap = np.zeros((self.nsuper, nchunk), np.int64)
        for s in range(self.nsuper):
            w0 = s * nsup
            for c in range(nchunk):
                off = 0
                for wl in range(nsup):
                    self.seg_off[w0 + wl, c] = off
                    off += caps[w0 + wl, c]
                self.piece_cap[s, c] = off
        # global slot base of each piece, pieces ordered (s, c)
        self.piece_base = np.zeros((self.nsuper, nchunk), np.int64)
        b = 0
        for s in range(self.nsuper):
            for c in range(nchunk):
                self.piece_base[s, c] = b
                b += self.piece_cap[s, c]
        self.total_slots = b
        self.w_has_edges = caps.sum(1) > 0

    def edge_slots(self, dstloc, srcflat):
        """Map per-core edges to absolute slots; returns (slot, idx16val, dloc)."""
        w = dstloc >> 7
        c = srcflat // CFG["CH"]
        s = w // self.nsup
        # piece-major, then subwindow, stable order
        key = (s * self.nchunk + c) * self.nsub + w
        order = np.argsort(key, kind="stable")
        ks = key[order]
        # rank within equal keys
        change = np.empty(len(ks), bool)
        if len(ks):
            change[0] = True
            change[1:] = ks[1:] != ks[:-1]
        starts = np.flatnonzero(change)
        rank = np.arange(len(ks)) - np.repeat(starts, np.diff(np.append(starts, len(ks))))
        wo, co, so = w[order], c[order], s[order]
        slot = self.piece_base[so, co] + self.seg_off[wo, co] + rank
        return order, slot


def _phase_structure(percore_edges, nsub, nsup, nchunk):
    """percore_edges: list of (dstloc, srcflat) -> caps [nsub, nchunk]."""
    ncore = len(percore_edges)
    cnts = np.zeros((ncore, nsub * nchunk), np.int64)
    for i, (dl, sf) in enumerate(percore_edges):
        seg = (dl >> 7) * nchunk + sf // CFG["CH"]
        cnts[i] = np.bincount(seg, minlength=nsub * nchunk)
    caps = cnts.max(0)
    caps = ((caps + 127) // 128) * 128
    return caps.reshape(nsub, nchunk)


def _pack_core_data(meta, dstloc, srcflat):
    """Returns idx16 [128, total/16] int16, dloc [128, total/128] f32."""
    T = meta.total_slots
    idxval = np.zeros(T, np.int16)
    dval = np.full(T, -1.0, np.float32)
    if len(dstloc):
        order, slot = meta.edge_slots(dstloc, srcflat)
        idxval[slot] = (srcflat[order] % CFG["CH"]).astype(np.int16)
        dval[slot] = (dstloc[order] & 127).astype(np.float32)
    # pack idx16: per 128-slot col a, slot e=a*128+j*16+cc -> [16cc, a*8+j]
    A = T // 128
    m = idxval.reshape(A * 8, 16).T                  # [16, A*8]
    idx16 = np.tile(m, (8, 1))                       # [128, A*8]
    dloc = dval.reshape(A, 128).T.copy()             # [128, A]
    return idx16, dloc


def _invcnt_pmajor(dstloc, nsub):
    cnt = np.bincount(dstloc, minlength=nsub * 128)
    inv = 1.0 / np.maximum(cnt, 1)
    return inv.reshape(nsub, 128).T.astype(np.float32).copy()


def _pmajor_rowmap(nsub):
    """global-local row r -> flat row p*nsub + w  of a [128, nsub, D] table."""

    def f(r):
        return (r % 128) * nsub + (r // 128)

    return f


# ---------------------------------------------------------------------------
# device kernel builder
# ---------------------------------------------------------------------------
def _emit_phase(nc, tile, pools, meta, src_view, idx_t, dloc_t, out_tile,
                invcnt_t=None, iota_t=None, e3=None, qstate=None, D=64):
    """Emit one gather/one-hot-reduce phase.  e3 = (vrep_tile, crep_tile)."""
    import concourse.mybir as mybir

    f32 = mybir.dt.float32
    CH = CFG["CH"]
    nsup = meta.nsup
    slotw = D if e3 is None else 2 * D
    slots_per_bank = 512 // slotw
    nbanks = (nsup + slots_per_bank - 1) // slots_per_bank
    OHG = CFG["OHG"]

    for s in range(meta.nsuper):
        banks = [pools["psum"].tile([128, 512], f32, tag="bank", name=f"bank{bi}")
                 for bi in range(nbanks)]
        for bk in banks:
            nc.vector.memset(bk[:], 0.0)

        def bank_slice(wl, lo, hi):
            b = wl // slots_per_bank
            off = (wl % slots_per_bank) * slotw
            return banks[b][:, off + lo:off + hi]

        # how many blocks feed each subwindow of this super (for start/stop)
        blk_total = {wl: int(meta.caps[s * nsup + wl, :].sum() // 128)
                     for wl in range(nsup)}
        blk_seen = {wl: 0 for wl in range(nsup)}

        for c in range(meta.nchunk):
            cap = int(meta.piece_cap[s, c])
            if cap == 0:
                continue
            A = cap // 128
            base = int(meta.piece_base[s, c])
            it = pools["idx"].tile([128, cap // 16], mybir.dt.int16, tag="idx")
            nc.sync.dma_start(out=it[:], in_=idx_t[:, base // 16:base // 16 + cap // 16])
            dl = pools["dloc"].tile([128, A], f32, tag="dloc")
            nc.sync.dma_start(out=dl[:], in_=dloc_t[:, base // 128:base // 128 + A])
            gt = pools["gather"].tile([128, A, D], f32, tag="gt")
            lo, hi = c * CH, min((c + 1) * CH, meta.table_rows)
            nc.gpsimd.dma_gather(
                out_ap=gt[:], in_ap=src_view[lo:hi], idxs_ap=it[:],
                num_idxs=cap, num_idxs_reg=cap, elem_size=D,
                queue_num=qstate[0] % CFG["NQ"], single_packet=False,
            )
            qstate[0] += 1

            if e3 is not None:
                vrep, crep = e3
                tmp = pools["tmp"].tile([128, A, D], f32, tag="tmp")
                nc.vector.tensor_tensor(
                    out=tmp[:], in0=gt[:],
                    in1=vrep[:].rearrange("p (o d) -> p o d", o=1).to_broadcast([128, A, D]),
                    op=mybir.AluOpType.mult)
                ze = pools["ze"].tile([128, A], f32, tag="ze")
                nc.vector.tensor_reduce(out=ze[:], in_=tmp[:],
                                        axis=mybir.AxisListType.X,
                                        op=mybir.AluOpType.add)
                nc.scalar.activation(out=ze[:], in_=ze[:],
                                     func=mybir.ActivationFunctionType.Exp,
                                     bias=crep[:, 0:1], scale=1.0)
                tmpb = pools["tmpb"].tile([128, A, D], mybir.dt.bfloat16,
                                          tag="tmpb")
                nc.vector.tensor_tensor(
                    out=tmpb[:], in0=gt[:],
                    in1=ze[:].rearrange("p (a o) -> p a o", o=1).to_broadcast([128, A, D]),
                    op=mybir.AluOpType.mult)
                zeb = pools["zeb"].tile([128, A], mybir.dt.bfloat16, tag="zeb")
                nc.vector.tensor_copy(out=zeb[:], in_=ze[:])
                rhs_feats, rhs_den = tmpb, zeb
            else:
                gtb = pools["tmpb"].tile([128, A, D], mybir.dt.bfloat16,
                                         tag="tmpb")
                nc.vector.tensor_copy(out=gtb[:], in_=gt[:])
                rhs_feats, rhs_den = gtb, None

            # one-hot groups + matmuls
            blocks = []  # (a, w_local)
            for wl in range(nsup):
                w = s * nsup + wl
                nb = int(meta.caps[w, c] // 128)
                off = int(meta.seg_off[w, c])
                for b in range(nb):
                    blocks.append(((off + b * 128) // 128, wl))
            blocks.sort()
            gi = 0
            while gi < len(blocks):
                g = blocks[gi:gi + OHG]
                a0 = g[0][0]
                ga = g[-1][0] - a0 + 1
                oh = pools["oh"].tile([128, OHG, 128], mybir.dt.bfloat16, tag="oh")
                nc.vector.tensor_tensor(
                    out=oh[:, :ga, :],
                    in0=iota_t[:].rearrange("p (o x) -> p o x", o=1).to_broadcast([128, ga, 128]),
                    in1=dl[:, a0:a0 + ga].rearrange("p (a o) -> p a o", o=1).to_broadcast([128, ga, 128]),
                    op=mybir.AluOpType.is_equal)
                for a, wl in g:
                    blk_seen[wl] += 1
                    last = blk_seen[wl] == blk_total[wl]
                    nc.tensor.matmul(
                        out=bank_slice(wl, 0, D), lhsT=oh[:, a - a0, :],
                        rhs=rhs_feats[:, a, :], start=False, stop=last,
                        skip_group_check=True)
                    if rhs_den is not None:
                        nc.tensor.matmul(
                            out=bank_slice(wl, D, D + 1), lhsT=oh[:, a - a0, :],
                            rhs=rhs_den[:, a:a + 1], start=False, stop=last,
                            skip_group_check=True)
                gi += len(g)

        # normalize + stage out
        stage = pools["stage"].tile([128, nsup * D], f32, tag="stage")
        if invcnt_t is not None:
            ic = pools["ic"].tile([128, nsup], f32, tag="ic")
            nc.sync.dma_start(out=ic[:], in_=invcnt_t[:, s * nsup:(s + 1) * nsup])
        for wl in range(nsup):
            w = s * nsup + wl
            dst = stage[:, wl * D:(wl + 1) * D]
            if not meta.w_has_edges[w]:
                nc.vector.memset(dst, 0.0)
                continue
            if e3 is None:
                nc.vector.tensor_scalar(
                    out=dst, in0=bank_slice(wl, 0, D),
                    scalar1=ic[:, wl:wl + 1], scalar2=None,
                    op0=mybir.AluOpType.mult)
            else:
                dt = pools["den"].tile([128, 1], f32, tag="den")
                nc.vector.tensor_scalar(
                    out=dt[:], in0=bank_slice(wl, D, D + 1),
                    scalar1=1e-9, scalar2=None, op0=mybir.AluOpType.max)
                nc.vector.reciprocal(out=dt[:], in_=dt[:])
                nc.vector.tensor_scalar(
                    out=dst, in0=bank_slice(wl, 0, D),
                    scalar1=dt[:, 0:1], scalar2=None,
                    op0=mybir.AluOpType.mult)
        nc.sync.dma_start(
            out=out_tile[:, s * nsup:(s + 1) * nsup, :],
            in_=stage[:].rearrange("p (w d) -> p w d", d=D))


def kernel(**inputs):
    _install_profile_hook()
    import concourse.bacc as bacc
    import concourse.mybir as mybir
    import concourse.tile as tile
    from concourse.bass_utils import run_bass_kernel_spmd

    f32 = mybir.dt.float32

    emb = np.asarray(inputs["emb_table"], np.float32)
    node_ids = np.asarray(inputs["node_ids"])
    w_o = np.asarray(inputs["w_o"], np.float32)
    b_o = np.asarray(inputs["b_o"], np.float32)
    att_w = np.asarray(inputs["att_w"], np.float32)
    att_b = np.asarray(inputs["att_b"], np.float32)
    e1_src = np.asarray(inputs["e1_src"], np.int64)
    e1_dst = np.asarray(inputs["e1_dst"], np.int64)
    e2_src = np.asarray(inputs["e2_src"], np.int64)
    e2_dst = np.asarray(inputs["e2_dst"], np.int64)
    e3_src = np.asarray(inputs["e3_src"], np.int64)
    e3_dst = np.asarray(inputs["e3_dst"], np.int64)

    N, D = emb.shape
    R, M, L = CFG["R"], CFG["M"], CFG["L"]
    NC, CH, SUB, NSUP, NSUP3 = (CFG["NCORE"], CFG["CH"], CFG["SUB"],
                                CFG["NSUP"], CFG["NSUP3"])

    x0 = emb[node_ids]                      # [N, D] (node_ids is arange per spec)
    v = (w_o @ att_w).astype(np.float32).ravel()          # [D]
    c_sc = float(b_o @ att_w.ravel() + att_b.ravel()[0])  # scalar

    NSH = N // NC
    MSH = M // NC
    nsub1 = -(-NSH // 128)
    nsub1 = -(-nsub1 // NSUP) * NSUP          # padded subwindows per core
    rows_x = NC * 128 * nsub1                 # p-major full-table rows
    map_x = _pmajor_rowmap(nsub1)

    nsub3 = -(-MSH // 128)
    nsub3 = -(-nsub3 // NSUP3) * NSUP3

    # ---------------- e1 edges per core (dst-range shard) -----------------
    core_of1 = np.minimum(e1_dst // NSH, NC - 1)
    e1_by_core = []
    for i in range(NC):
        m = core_of1 == i
        e1_by_core.append((e1_dst[m] - i * NSH, e1_src[m]))
    caps1_l1 = _phase_structure([(d, s) for d, s in e1_by_core], nsub1, NSUP,
                                -(-N // CH))
    meta1_l1 = PhaseMeta(nsub1, NSUP, -(-N // CH), N, caps1_l1)
    # layers 2..L gather from p-major x tables
    e1_by_core_pm = []
    for d, s in e1_by_core:
        ci = np.minimum(s // NSH, NC - 1)
        r = s - ci * NSH
        flat = (ci * 128 + (r % 128)) * nsub1 + r // 128
        e1_by_core_pm.append((d, flat))
    caps1_lx = _phase_structure(e1_by_core_pm, nsub1, NSUP, -(-rows_x // CH))
    meta1_lx = PhaseMeta(nsub1, NSUP, -(-rows_x // CH), rows_x, caps1_lx)

    # ---------------- e2: consumer-sharded reviews ------------------------
    e2cnt = np.bincount(e2_dst, minlength=R)          # global review in-degree
    core_of3 = np.minimum(e3_dst // MSH, NC - 1)
    # chunk signature ordering for packing (based on xbar p-major rows)
    ci2 = np.minimum(e2_src // NSH, NC - 1)
    r2 = e2_src - ci2 * NSH
    e2_srcflat = (ci2 * 128 + (r2 % 128)) * nsub1 + r2 // 128
    e2_chunk = e2_srcflat // CH

    cons_lists, e2_data, e3_data, inv2_list = [], [], [], []
    # per-review (min, max) source chunk — sort key for pack-friendly numbering
    o2 = np.lexsort((e2_chunk, e2_dst))
    e2d_s, e2c_s = e2_dst[o2], e2_chunk[o2]
    rstart = np.searchsorted(e2d_s, np.arange(R + 1))
    cmin = np.full(R, 99, np.int64)
    cmax = np.full(R, 99, np.int64)
    has = rstart[1:] > rstart[:-1]
    if len(e2c_s):
        cmin[has] = e2c_s[rstart[:-1][has]]
        cmax[has] = e2c_s[rstart[1:][has] - 1]

    for i in range(NC):
        m3 = core_of3 == i
        src3 = e3_src[m3]
        dst3 = e3_dst[m3] - i * MSH
        cons = np.unique(src3)
        # pack-friendly ordering: by (cmin, cmax) of each review's e2 edges
        key = cmin[cons].astype(np.int64) * 100 + cmax[cons]
        cons = cons[np.argsort(key, kind="stable")]
        lid = np.full(R, -1, np.int64)
        lid[cons] = np.arange(len(cons))
        cons_lists.append(cons)
        sel = lid[e2_dst] >= 0
        e2_data.append((lid[e2_dst[sel]], e2_srcflat[sel]))
        e3_data.append((dst3, lid[src3]))
        inv2 = 1.0 / np.maximum(e2cnt[cons], 1)
        inv2_list.append(inv2.astype(np.float32))

    revcap = max(len(c) for c in cons_lists)
    nsub2 = -(-revcap // 128)
    nsub2 = -(-nsub2 // NSUP) * NSUP
    rows_rev = 128 * nsub2
    map_rev = _pmajor_rowmap(nsub2)

    caps2 = _phase_structure(e2_data, nsub2, NSUP, -(-rows_x // CH))
    meta2 = PhaseMeta(nsub2, NSUP, -(-rows_x // CH), rows_x, caps2)

    e3_data_pm = [(d, map_rev(s)) for d, s in e3_data]
    caps3 = _phase_structure(e3_data_pm, nsub3, NSUP3, -(-rows_rev // CH))
    meta3 = PhaseMeta(nsub3, NSUP3, -(-rows_rev // CH), rows_rev, caps3)

    # ---------------- per-core input arrays -------------------------------
    in_maps = []
    for i in range(NC):
        d1, s1 = e1_by_core[i]
        idxA, dlA = _pack_core_data(meta1_l1, d1, s1)
        d1x, s1x = e1_by_core_pm[i]
        idxB, dlB = _pack_core_data(meta1_lx, d1x, s1x)
        inv1 = _invcnt_pmajor(d1, nsub1)
        d2, s2 = e2_data[i]
        idx2, dl2 = _pack_core_data(meta2, d2, s2)
        inv2 = np.zeros((128, nsub2), np.float32)
        li = np.arange(len(cons_lists[i]))
        inv2[li % 128, li // 128] = inv2_list[i]
        d3, s3 = e3_data_pm[i]
        idx3, dl3 = _pack_core_data(meta3, d3, s3)
        # emb_local p-major [128, nsub1, D]
        embl = np.zeros((128, nsub1, D), np.float32)
        loc = x0[i * NSH:(i + 1) * NSH]
        r = np.arange(NSH)
        embl[r % 128, r // 128] = loc
        in_maps.append({
            "emb": np.ascontiguousarray(x0),
            "emb_local": embl,
            "idx_l1": idxA, "dl_l1": dlA,
            "idx_lx": idxB, "dl_lx": dlB,
            "inv1": inv1,
            "idx_e2": idx2, "dl_e2": dl2, "inv2": inv2,
            "idx_e3": idx3, "dl_e3": dl3,
            "iota": np.tile(np.arange(128, dtype=np.float32), (128, 1)),
            "vrep": np.tile(v, (128, 1)).astype(np.float32),
            "crep": np.full((128, 1), c_sc, np.float32),
        })

    # ---------------- build device program --------------------------------
    nc = bacc.Bacc("TRN2", target_bir_lowering=False, debug=False,
                   num_devices=NC, num_swdge_queues=CFG["NQ"])

    def din(name, arr):
        return nc.dram_tensor(name, list(arr.shape),
                              mybir.dt.from_np(arr.dtype), kind="ExternalInput")

    t = {k: din(k, in_maps[0][k]) for k in in_maps[0]}
    out_t = nc.dram_tensor("out", [128, nsub3, D], f32, kind="ExternalOutput")
    dbg_t = {}
    if CFG.get("DEBUG"):
        for nm, sh in (("d_x1", [128, nsub1, D]), ("d_x2", [128, nsub1, D]),
                       ("d_x3", [128, nsub1, D]), ("d_xbar", [128, nsub1, D]),
                       ("d_rev", [128, nsub2, D])):
            dbg_t[nm] = nc.dram_tensor(nm, sh, f32, kind="ExternalOutput")

    qstate = [0]
    with tile.TileContext(nc) as tc:
        with (
            tc.tile_pool(name="psum", bufs=6, space="PSUM") as psum_p,
            tc.tile_pool(name="gather", bufs=5) as gather_p,
            tc.tile_pool(name="idx", bufs=5) as idx_p,
            tc.tile_pool(name="dloc", bufs=5) as dloc_p,
            tc.tile_pool(name="oh", bufs=4) as oh_p,
            tc.tile_pool(name="stage", bufs=3) as stage_p,
            tc.tile_pool(name="ic", bufs=3) as ic_p,
            tc.tile_pool(name="tmp", bufs=3) as tmp_p,
            tc.tile_pool(name="tmpb", bufs=4) as tmpb_p,
            tc.tile_pool(name="zeb", bufs=3) as zeb_p,
            tc.tile_pool(name="ze", bufs=3) as ze_p,
            tc.tile_pool(name="den", bufs=4) as den_p,
            tc.tile_pool(name="const", bufs=1) as const_p,
            tc.tile_pool(name="ro", bufs=4) as ro_p,
            tc.tile_pool(name="dram", bufs=1, space="DRAM") as dram_p,
        ):
            pools = {"psum": psum_p, "gather": gather_p, "idx": idx_p,
                     "dloc": dloc_p, "oh": oh_p, "stage": stage_p,
                     "ic": ic_p, "tmp": tmp_p, "ze": ze_p, "den": den_p,
                     "tmpb": tmpb_p, "zeb": zeb_p}
            iota_t = const_p.tile([128, 128], f32, tag="iota")
            nc.sync.dma_start(out=iota_t[:], in_=t["iota"][:])
            vrep_t = const_p.tile([128, D], f32, tag="vrep")
            nc.sync.dma_start(out=vrep_t[:], in_=t["vrep"][:])
            crep_t = const_p.tile([128, 1], f32, tag="crep")
            nc.sync.dma_start(out=crep_t[:], in_=t["crep"][:])

            x_loc = [dram_p.tile([128, nsub1, D], f32, tag="x_loc", name=f"x_loc{l}") for l in range(L)]
            x_full = [dram_p.tile([NC * 128, nsub1, D], f32, tag="x_full", name=f"x_full{l}")
                      for l in range(L - 1)]
            xbar_loc = dram_p.tile([128, nsub1, D], f32, tag="xbar_loc", name="xbar_loc")
            xbar_full = dram_p.tile([NC * 128, nsub1, D], f32, tag="xbar_full", name="xbar_full")
            rev_loc = dram_p.tile([128, nsub2, D], f32, tag="rev_loc", name="rev_loc")

            # ---- propagation layers ----
            for l in range(L):
                if l == 0:
                    src_view = t["emb"][:]
                    meta_l = meta1_l1
                    idx_l, dl_l = t["idx_l1"], t["dl_l1"]
                else:
                    src_view = x_full[l - 1][:].rearrange("a w d -> (a w) d")
                    meta_l = meta1_lx
                    idx_l, dl_l = t["idx_lx"], t["dl_lx"]
                _emit_phase(nc, tile, pools, meta_l, src_view,
                            idx_l[:], dl_l[:], x_loc[l],
                            invcnt_t=t["inv1"][:], iota_t=iota_t,
                            qstate=qstate, D=D)
                if l < L - 1:
                    nc.gpsimd.collective_compute(
                        "AllGather", mybir.AluOpType.bypass,
                        replica_groups=[list(range(NC))],
                        ins=[x_loc[l].opt()], outs=[x_full[l].opt()])

            # ---- readout mean ----
            RT = 16
            for w0 in range(0, nsub1, RT):
                wn = min(RT, nsub1 - w0)
                acc = ro_p.tile([128, RT, D], f32, tag="roacc")
                nc.sync.dma_start(out=acc[:, :wn, :],
                                  in_=t["emb_local"][:, w0:w0 + wn, :])
                for l in range(L):
                    tl = ro_p.tile([128, RT, D], f32, tag="rold")
                    nc.sync.dma_start(out=tl[:, :wn, :],
                                      in_=x_loc[l][:, w0:w0 + wn, :])
                    nc.vector.tensor_tensor(out=acc[:, :wn, :],
                                            in0=acc[:, :wn, :],
                                            in1=tl[:, :wn, :],
                                            op=mybir.AluOpType.add)
                nc.vector.tensor_scalar(out=acc[:, :wn, :], in0=acc[:, :wn, :],
                                        scalar1=1.0 / (L + 1), scalar2=None,
                                        op0=mybir.AluOpType.mult)
                nc.sync.dma_start(out=xbar_loc[:, w0:w0 + wn, :],
                                  in_=acc[:, :wn, :])
            nc.gpsimd.collective_compute(
                "AllGather", mybir.AluOpType.bypass,
                replica_groups=[list(range(NC))],
                ins=[xbar_loc.opt()], outs=[xbar_full.opt()])

            # ---- e2: review representations ----
            _emit_phase(nc, tile, pools, meta2,
                        xbar_full[:].rearrange("a w d -> (a w) d"),
                        t["idx_e2"][:], t["dl_e2"][:], rev_loc,
                        invcnt_t=t["inv2"][:], iota_t=iota_t,
                        qstate=qstate, D=D)

            if CFG.get("DEBUG"):
                for nm, srcv in (("d_x1", x_loc[0]), ("d_x2", x_loc[1]),
                                 ("d_x3", x_loc[2]), ("d_xbar", xbar_loc),
                                 ("d_rev", rev_loc)):
                    nsb = srcv.shape[1]
                    for w0 in range(0, nsb, 16):
                        wn = min(16, nsb - w0)
                        bt = ro_p.tile([128, 16, D], f32, tag="dbgb",
                                       name=f"dbgb_{nm}_{w0}")
                        nc.sync.dma_start(out=bt[:, :wn, :],
                                          in_=srcv[:, w0:w0 + wn, :])
                        nc.sync.dma_start(out=dbg_t[nm][:, w0:w0 + wn, :],
                                          in_=bt[:, :wn, :])

            # ---- e3: edge-softmax attention ----
            _emit_phase(nc, tile, pools, meta3,
                        rev_loc[:].rearrange("p w d -> (p w) d"),
                        t["idx_e3"][:], t["dl_e3"][:], out_t,
                        invcnt_t=None, iota_t=iota_t,
                        e3=(vrep_t, crep_t), qstate=qstate, D=D)

    nc.compile()

    res = run_bass_kernel_spmd(nc, in_maps, core_ids=list(range(NC)),
                               trace=CFG["TRACE"] or os.environ.get("GNN_TRACE") == "1")
    _LAST["exec_ns"] = res.exec_time_ns
    _LAST["profile_json"] = res.profile_json
    _LAST["results"] = res.results

    out = np.empty((M, D), np.float32)
    for i in range(NC):
        o = res.results[i]["out"]          # [128, nsub3, D]
        r = np.arange(MSH)
        out[i * MSH:(i + 1) * MSH] = o[r % 128, r // 128]
    return out



# revision 15
# speedup vs baseline: 2.2776x; 2.2776x over previous
"""LightGCN-style GNN (3 mean-agg layers + review conv + edge-softmax attention)
on 8 Trainium2 NeuronCores.

v2 design (vs. baseline):
  * dst-row sharding with w-major (window-major) table layouts so each
    src chunk (32768 rows, int16-addressable) aligns exactly with one
    AllGather piece -> per-piece AllGathers pipeline with compute.
  * exact (non-128-rounded) per-cell capacities: gather descriptor count
    drops ~25% (the Q7 SWDGE descriptor emission is the kernel bottleneck
    at ~7.6ns/descriptor).  128-slot columns may span multiple dst
    windows; a host-built (column, window) j-map drives one masked
    one-hot + matmul per pair.
  * e1 layers accumulate per-piece PSUM into per-super SBUF f32
    accumulators so pieces from different chunks/supers pipeline freely.
  * f32->bf16 casts run on the idle Scalar (ACT) engine, not DVE.
  * AllGather outputs are addr_space="Shared" (fast collective path).
"""

import os
import sys
import types

import numpy as np

CFG = {
    "R": 400_000,
    "M": 100_000,
    "L": 3,
    "NCORE": 8,
    "CH": 32768,
    "OHG": 16,         # one-hot columns per DVE is_equal op
    "NQ": 4,           # SWDGE queues
    "TRACE": False,
}

_LAST = {"exec_ns": None, "profile_json": None}


def _install_profile_hook():
    try:
        if "antenv.axon_hooks" in sys.modules:
            return
        import antenv

        mod = types.ModuleType("antenv.axon_hooks")
        mod._hook = None
        mod.set_axon_ntff_profile_hook = lambda h: setattr(mod, "_hook", h)
        mod.get_axon_ntff_profile_hook = lambda: mod._hook
        sys.modules["antenv.axon_hooks"] = mod
        antenv.axon_hooks = mod
        from trn_agent_boot.trn_boot import _ntff_profile_via_ctypes

        mod.set_axon_ntff_profile_hook(
            _ntff_profile_via_ctypes("/opt/axon/libaxon_pjrt.so")
        )
    except Exception:
        pass


# ---------------------------------------------------------------------------
# host-side index preparation
# ---------------------------------------------------------------------------
class Meta:
    """Static structure of one gather/reduce phase (w-major chunked source).

    Slot space: pieces ordered (super, chunk); piece slots = concat of the
    per-window exact cell capacities (max over cores), piece total rounded
    up to 128 with dummy slots.  jmap per piece: (column, window) pairs.
    """

    def __init__(self, nsub, nsup, chunk_rows, percore_cells):
        # percore_cells: [ncore, nsub, nchunk] int counts
        self.nsub, self.nsup = nsub, nsup
        self.nchunk = len(chunk_rows)
        self.chunk_rows = chunk_rows
        self.nsuper = nsub // nsup
        cells = percore_cells.max(0)                  # [nsub, nchunk]
        self.cells = cells
        self.w_has_edges = cells.sum(1) > 0
        self.pieces = {}                              # (s, c) -> dict
        idxbase = 0
        jbase = 0
        # static slot labels (window of each slot; -1 for dummy pad)
        wlab_parts = []
        # per-cell slot base for edge->slot mapping
        self.cell_base = np.full((nsub, self.nchunk), -1, np.int64)
        for s in range(self.nsuper):
            for c in range(self.nchunk):
                segs = cells[s * nsup:(s + 1) * nsup, c]
                cap = int(segs.sum())
                if cap == 0:
                    continue
                capR = -(-cap // 128) * 128
                A = capR // 128
                off = np.concatenate([[0], np.cumsum(segs)])
                for wl in range(nsup):
                    self.cell_base[s * nsup + wl, c] = idxbase + off[wl]
                wlab = np.full(capR, -1, np.int64)
                for wl in range(nsup):
                    wlab[off[wl]:off[wl + 1]] = wl
                wlab_parts.append(wlab)
                jmap = []
                for a in range(A):
                    lo, hi = a * 128, min((a + 1) * 128, cap)
                    for wl in range(nsup):
                        if off[wl] < hi and off[wl + 1] > lo:
                            jmap.append((a, wl))
                self.pieces[(s, c)] = dict(
                    cap=capR, A=A, jmap=jmap, idxbase=idxbase, jbase=jbase,
                    wls=sorted(set(wl for _, wl in jmap)),
                )
                idxbase += capR
                jbase += len(jmap)
        self.tot_idx = idxbase
        self.tot_j = jbase
        self.wlab = (np.concatenate(wlab_parts) if wlab_parts
                     else np.zeros(0, np.int64))
        # j -> slot base / window arrays for vectorized dloc_exp build
        jsb = np.zeros(jbase, np.int64)
        jwl = np.zeros(jbase, np.int64)
        for p in self.pieces.values():
            for k, (a, wl) in enumerate(p["jmap"]):
                jsb[p["jbase"] + k] = p["idxbase"] + a * 128
                jwl[p["jbase"] + k] = wl
        self.jsb, self.jwl = jsb, jwl

    def pack(self, dstloc, chunk, idx):
        """Per-core edge data -> (idx16 [128, tot/16], dloc_exp [128, totj])."""
        T = self.tot_idx
        idxval = np.zeros(T, np.int16)     # dummy slots gather row 0
        dval = np.full(T, -1.0, np.float32)
        if len(dstloc):
            w = dstloc >> 7
            key = ((w // self.nsup) * self.nchunk + chunk) * self.nsub + w
            order = np.argsort(key, kind="stable")
            ks = key[order]
            change = np.empty(len(ks), bool)
            change[0] = True
            change[1:] = ks[1:] != ks[:-1]
            starts = np.flatnonzero(change)
            rank = np.arange(len(ks)) - np.repeat(
                starts, np.diff(np.append(starts, len(ks))))
            slot = self.cell_base[w[order], chunk[order]] + rank
            idxval[slot] = idx[order].astype(np.int16)
            dval[slot] = (dstloc[order] & 127).astype(np.float32)
        m = idxval.reshape(T // 16, 16).T           # [16, T/16]
        idx16 = np.tile(m, (8, 1))                  # [128, T/16]
        # dloc_exp: [128, totj]
        rows = self.jsb[None, :] + np.arange(128)[:, None]   # [128, J]
        dexp = np.where(self.wlab[rows] == self.jwl[None, :],
                        dval[rows], -1.0).astype(np.float32)
        return idx16, dexp


def _cells_of(percore, nsub, nchunk):
    ncore = len(percore)
    cnts = np.zeros((ncore, nsub, nchunk), np.int64)
    for i, (dl, c, ix) in enumerate(percore):
        seg = (dl >> 7) * nchunk + c
        cnts[i] = np.bincount(seg, minlength=nsub * nchunk).reshape(nsub, nchunk)
    return cnts


def _wmajor_src(src, NSH, nsub_src, wpc_list):
    """Global src node id -> (chunk, idx-within-chunk) in w-major AG layout."""
    ci = src // NSH
    r = src - ci * NSH
    w = r // 128
    p = r & 127
    bounds = np.cumsum([0] + wpc_list)
    c = np.searchsorted(bounds, w, side="right") - 1
    wl = w - bounds[c]
    wpc = np.asarray(wpc_list)[c]
    idx = ci * (wpc * 128) + wl * 128 + p
    return c, idx


# ---------------------------------------------------------------------------
# device phase emitters
# ---------------------------------------------------------------------------
def _emit_piece_gather(nc, pools, meta, piece, src_view, idx_t, dexp_t, qstate):
    """DMA idx/dloc, gather, cast to bf16, build one-hots. Returns tiles."""
    import concourse.mybir as mybir

    f32 = mybir.dt.float32
    cap, A, J = piece["cap"], piece["A"], len(piece["jmap"])
    ib, jb = piece["idxbase"], piece["jbase"]
    it = pools["idx"].tile([128, cap // 16], mybir.dt.int16, tag="idx")
    nc.sync.dma_start(out=it[:], in_=idx_t[:, ib // 16: ib // 16 + cap // 16])
    dlt = pools["dloc"].tile([128, J], f32, tag="dloc")
    nc.sync.dma_start(out=dlt[:], in_=dexp_t[:, jb:jb + J])
    gt = pools["gather"].tile([128, A, 64], f32, tag="gt")
    nc.gpsimd.dma_gather(
        out_ap=gt[:], in_ap=src_view, idxs_ap=it[:],
        num_idxs=cap, num_idxs_reg=cap, elem_size=64,
        queue_num=qstate[0] % CFG["NQ"], single_packet=False,
    )
    qstate[0] += 1
    return gt, dlt


def _emit_onehots(nc, pools, piece, dlt, iota_t):
    """Build all J one-hot columns for a piece (groups of OHG)."""
    import concourse.mybir as mybir

    J = len(piece["jmap"])
    OHG = CFG["OHG"]
    ohs = []
    for j0 in range(0, J, OHG):
        g = min(OHG, J - j0)
        oh = pools["oh"].tile([128, OHG, 128], mybir.dt.bfloat16, tag="oh")
        nc.vector.tensor_tensor(
            out=oh[:, :g, :],
            in0=iota_t[:].rearrange("p (o x) -> p o x", o=1).to_broadcast([128, g, 128]),
            in1=dlt[:, j0:j0 + g].rearrange("p (a o) -> p a o", o=1).to_broadcast([128, g, 128]),
            op=mybir.AluOpType.is_equal)
        ohs.append(oh)
    return ohs


def _runs(wls):
    """Consecutive runs in a sorted window list: [(w0, w1), ...)."""
    runs = []
    for w in wls:
        if runs and runs[-1][1] == w:
            runs[-1][1] = w + 1
        else:
            runs.append([w, w + 1])
    return runs




def _e1_chunks(nc, mybir, pools, meta, stacc_s, s, srcv, t, qstate,
               chunks, memset_first, NSUP, D):
    f32 = mybir.dt.float32
    bf16 = mybir.dt.bfloat16
    if memset_first:
        nc.vector.memset(stacc_s[:], 0.0)
    for c in chunks:
        piece = meta.pieces.get((s, c))
        if piece is None:
            continue
        gt, dlt = _emit_piece_gather(nc, pools, meta, piece, srcv[c],
                                     t["idx_e1"], t["dx_e1"], qstate)
        gtb = pools["gatherb"].tile([128, piece["A"], D], bf16, tag="gtb")
        nc.scalar.activation(out=gtb[:], in_=gt[:],
                             func=mybir.ActivationFunctionType.Copy)
        ohs = _emit_onehots(nc, pools, piece, dlt, t["iota_t"])
        ps = pools["psum"].tile([128, NSUP, D], f32, tag="pp")
        seen, total = {}, {}
        for _, wl in piece["jmap"]:
            total[wl] = total.get(wl, 0) + 1
        for k, (a, wl) in enumerate(piece["jmap"]):
            seen[wl] = seen.get(wl, 0) + 1
            nc.tensor.matmul(
                out=ps[:, wl, :],
                lhsT=ohs[k // CFG["OHG"]][:, k % CFG["OHG"], :],
                rhs=gtb[:, a, :],
                start=(seen[wl] == 1), stop=(seen[wl] == total[wl]),
                skip_group_check=True)
        for w0, w1 in _runs(piece["wls"]):
            nc.vector.tensor_tensor(
                out=stacc_s[:, w0:w1, :], in0=stacc_s[:, w0:w1, :],
                in1=ps[:, w0:w1, :], op=mybir.AluOpType.add)


def _e1_norm(nc, mybir, pools, stacc_s, s, t, x_loc_l, NSUP, D):
    f32 = mybir.dt.float32
    ic = pools["ic"].tile([128, NSUP], f32, tag="ic")
    nc.sync.dma_start(out=ic[:], in_=t["inv1"][:, s * NSUP:(s + 1) * NSUP])
    st = pools["stage"].tile([128, NSUP, D], f32, tag="st")
    nc.vector.tensor_tensor(
        out=st[:], in0=stacc_s[:],
        in1=ic[:].rearrange("p (w o) -> p w o", o=1).to_broadcast(
            [128, NSUP, D]),
        op=mybir.AluOpType.mult)
    nc.sync.dma_start(
        out=x_loc_l[s * NSUP:(s + 1) * NSUP].rearrange("w p d -> p w d"),
        in_=st[:])


def _readout_super(nc, mybir, pools, s, t, x_loc, xbar_loc, NSUP, D, L):
    f32 = mybir.dt.float32
    sl = slice(s * NSUP, (s + 1) * NSUP)
    acc = pools["ro"].tile([128, NSUP, D], f32, tag="roacc")
    nc.sync.dma_start(out=acc[:],
                      in_=t["emb_local"][sl].rearrange("w p d -> p w d"))
    for l in range(L):
        tl = pools["ro"].tile([128, NSUP, D], f32, tag="rold")
        nc.sync.dma_start(out=tl[:],
                          in_=x_loc[l][sl].rearrange("w p d -> p w d"))
        nc.vector.tensor_tensor(out=acc[:], in0=acc[:], in1=tl[:],
                                op=mybir.AluOpType.add)
    nc.vector.tensor_scalar(out=acc[:], in0=acc[:],
                            scalar1=1.0 / (L + 1), scalar2=None,
                            op0=mybir.AluOpType.mult)
    nc.sync.dma_start(out=xbar_loc[sl].rearrange("w p d -> p w d"),
                      in_=acc[:])


def _e2_super(nc, mybir, pools, meta, s, srcv, t, qstate, rev_loc, NSUP, D,
              nch):
    f32 = mybir.dt.float32
    bf16 = mybir.dt.bfloat16
    psb = pools["psum2"].tile([128, NSUP * D], f32, tag="ppx", name="ppx2")
    nc.vector.memset(psb[:], 0.0)
    ps = psb[:].rearrange("p (w d) -> p w d", d=D)
    seen, total = {}, {}
    for c in range(nch):
        piece = meta.pieces.get((s, c))
        if piece is None:
            continue
        for _, wl in piece["jmap"]:
            total[wl] = total.get(wl, 0) + 1
    for c in range(nch):
        piece = meta.pieces.get((s, c))
        if piece is None:
            continue
        gt, dlt = _emit_piece_gather(nc, pools, meta, piece, srcv[c],
                                     t["idx_e2"], t["dx_e2"], qstate)
        gtb = pools["gatherb"].tile([128, piece["A"], D], bf16, tag="gtb")
        nc.scalar.activation(out=gtb[:], in_=gt[:],
                             func=mybir.ActivationFunctionType.Copy)
        ohs = _emit_onehots(nc, pools, piece, dlt, t["iota_t"])
        for k, (a, wl) in enumerate(piece["jmap"]):
            seen[wl] = seen.get(wl, 0) + 1
            nc.tensor.matmul(
                out=ps[:, wl, :],
                lhsT=ohs[k // CFG["OHG"]][:, k % CFG["OHG"], :],
                rhs=gtb[:, a, :],
                start=False, stop=(seen[wl] == total[wl]),
                skip_group_check=True)
    ic = pools["ic"].tile([128, NSUP], f32, tag="ic2")
    nc.sync.dma_start(out=ic[:], in_=t["inv2"][:, s * NSUP:(s + 1) * NSUP])
    st = pools["stage"].tile([128, NSUP, D], f32, tag="st2")
    nc.vector.tensor_tensor(
        out=st[:], in0=ps[:],
        in1=ic[:].rearrange("p (w o) -> p w o", o=1).to_broadcast(
            [128, NSUP, D]),
        op=mybir.AluOpType.mult)
    nc.sync.dma_start(
        out=rev_loc[s * NSUP:(s + 1) * NSUP].rearrange("w p d -> p w d"),
        in_=st[:])


def _e3_super(nc, mybir, pools, meta, s, srcv, t, qstate, out_t,
              vrep_t, crep_t, iota_t, NSUP3, D, nch):
    f32 = mybir.dt.float32
    bf16 = mybir.dt.bfloat16
    psb = pools["psum2"].tile([128, NSUP3 * 2 * D], f32, tag="ppx", name="ppx3")
    nc.vector.memset(psb[:], 0.0)
    ps = psb[:].rearrange("p (w d) -> p w d", d=2 * D)
    seen, total = {}, {}
    for c in range(nch):
        piece = meta.pieces.get((s, c))
        if piece is None:
            continue
        for _, wl in piece["jmap"]:
            total[wl] = total.get(wl, 0) + 1
    for c in range(nch):
        piece = meta.pieces.get((s, c))
        if piece is None:
            continue
        gt, dlt = _emit_piece_gather(nc, pools, meta, piece, srcv[c],
                                     t["idx_e3"], t["dx_e3"], qstate)
        A = piece["A"]
        tmp = pools["gather"].tile([128, A, D], f32, tag="tmp3")
        nc.vector.tensor_tensor(
            out=tmp[:], in0=gt[:],
            in1=vrep_t[:].rearrange("p (o d) -> p o d", o=1).to_broadcast(
                [128, A, D]),
            op=mybir.AluOpType.mult)
        ze = pools["ze"].tile([128, A], f32, tag="ze")
        nc.vector.tensor_reduce(out=ze[:], in_=tmp[:],
                                axis=mybir.AxisListType.X,
                                op=mybir.AluOpType.add)
        nc.scalar.activation(out=ze[:], in_=ze[:],
                             func=mybir.ActivationFunctionType.Exp,
                             bias=crep_t[:, 0:1], scale=1.0)
        tmpb = pools["tmpb"].tile([128, A, D], bf16, tag="tmpb")
        nc.vector.tensor_tensor(
            out=tmpb[:], in0=gt[:],
            in1=ze[:].rearrange("p (a o) -> p a o", o=1).to_broadcast(
                [128, A, D]),
            op=mybir.AluOpType.mult)
        zeb = pools["zeb"].tile([128, A], bf16, tag="zeb")
        nc.vector.tensor_copy(out=zeb[:], in_=ze[:])
        ohs = _emit_onehots(nc, pools, piece, dlt, iota_t)
        for k, (a, wl) in enumerate(piece["jmap"]):
            seen[wl] = seen.get(wl, 0) + 1
            oh = ohs[k // CFG["OHG"]][:, k % CFG["OHG"], :]
            last = seen[wl] == total[wl]
            nc.tensor.matmul(out=ps[:, wl, 0:D], lhsT=oh, rhs=tmpb[:, a, :],
                             start=False, stop=last, skip_group_check=True)
            nc.tensor.matmul(out=ps[:, wl, D:D + 1], lhsT=oh,
                             rhs=zeb[:, a:a + 1],
                             start=False, stop=last, skip_group_check=True)
    st = pools["stage"].tile([128, NSUP3, D], f32, tag="st3")
    dt = pools["den"].tile([128, NSUP3, 1], f32, tag="den")
    nc.vector.tensor_scalar(out=dt[:], in0=ps[:, :, D:D + 1],
                            scalar1=1e-9, scalar2=None,
                            op0=mybir.AluOpType.max)
    nc.vector.reciprocal(out=dt[:], in_=dt[:])
    nc.vector.tensor_tensor(out=st[:], in0=ps[:, :, 0:D],
                            in1=dt[:].to_broadcast([128, NSUP3, D]),
                            op=mybir.AluOpType.mult)
    nc.sync.dma_start(
        out=out_t[s * NSUP3:(s + 1) * NSUP3].rearrange("w p d -> p w d"),
        in_=st[:])

def kernel(**inputs):
    _install_profile_hook()
    import concourse.bacc as bacc
    import concourse.mybir as mybir
    import concourse.tile as tile
    from concourse.bass_utils import run_bass_kernel_spmd

    f32 = mybir.dt.float32
    bf16 = mybir.dt.bfloat16

    emb = np.asarray(inputs["emb_table"], np.float32)
    node_ids = np.asarray(inputs["node_ids"])
    w_o = np.asarray(inputs["w_o"], np.float32)
    b_o = np.asarray(inputs["b_o"], np.float32)
    att_w = np.asarray(inputs["att_w"], np.float32)
    att_b = np.asarray(inputs["att_b"], np.float32)
    e1_src = np.asarray(inputs["e1_src"], np.int64)
    e1_dst = np.asarray(inputs["e1_dst"], np.int64)
    e2_src = np.asarray(inputs["e2_src"], np.int64)
    e2_dst = np.asarray(inputs["e2_dst"], np.int64)
    e3_src = np.asarray(inputs["e3_src"], np.int64)
    e3_dst = np.asarray(inputs["e3_dst"], np.int64)

    N, D = emb.shape
    R, M, L = CFG["R"], CFG["M"], CFG["L"]
    NC, CH = CFG["NCORE"], CFG["CH"]

    x0 = emb[node_ids]
    v = (w_o @ att_w).astype(np.float32).ravel()
    c_sc = float(b_o @ att_w.ravel() + att_b.ravel()[0])

    NSH = N // NC
    MSH = M // NC
    NSUP = 16

    nsub1 = -(-NSH // 128)
    nsub1 = -(-nsub1 // NSUP) * NSUP            # 208
    wpc1 = []
    w = nsub1
    while w > 0:
        wpc1.append(min(32, w))
        w -= 32
    nch1 = len(wpc1)                            # 7
    chunk_rows1 = [NC * wp * 128 for wp in wpc1]

    # ---------------- e1 (shared meta for all 3 layers) -------------------
    core1 = np.minimum(e1_dst // NSH, NC - 1)
    e1_pc = []
    for i in range(NC):
        m = core1 == i
        d = e1_dst[m] - i * NSH
        c, ix = _wmajor_src(e1_src[m], NSH, nsub1, wpc1)
        e1_pc.append((d, c, ix))
    meta1 = Meta(nsub1, NSUP, chunk_rows1, _cells_of(e1_pc, nsub1, nch1))

    # ---------------- e2: consumer-sharded reviews ------------------------
    e2cnt = np.bincount(e2_dst, minlength=R)
    core3 = np.minimum(e3_dst // MSH, NC - 1)
    e2_chunk = e2_src // NSH
    e2_idx = e2_src - e2_chunk * NSH

    # pack-friendly review ordering: by (min, max) e2 source chunk
    o2 = np.lexsort((e2_chunk, e2_dst))
    e2d_s, e2c_s = e2_dst[o2], e2_chunk[o2]
    rstart = np.searchsorted(e2d_s, np.arange(R + 1))
    cmin = np.full(R, 99, np.int64)
    cmax = np.full(R, 99, np.int64)
    has = rstart[1:] > rstart[:-1]
    if len(e2c_s):
        cmin[has] = e2c_s[rstart[:-1][has]]
        cmax[has] = e2c_s[rstart[1:][has] - 1]

    cons_lists, e2_data, e3_data, inv2_list = [], [], [], []
    for i in range(NC):
        m3 = core3 == i
        src3 = e3_src[m3]
        dst3 = e3_dst[m3] - i * MSH
        cons = np.unique(src3)
        key = cmin[cons] * 100 + cmax[cons]
        cons = cons[np.argsort(key, kind="stable")]
        lid = np.full(R, -1, np.int64)
        lid[cons] = np.arange(len(cons))
        cons_lists.append(cons)
        sel = lid[e2_dst] >= 0
        e2_data.append((lid[e2_dst[sel]], e2_chunk[sel], e2_idx[sel]))
        e3_data.append((dst3, lid[src3]))
        inv2_list.append((1.0 / np.maximum(e2cnt[cons], 1)).astype(np.float32))

    revcap = max(len(c) for c in cons_lists)
    nsub2 = -(-revcap // 128)
    nsub2 = -(-nsub2 // NSUP) * NSUP
    nch2 = NC
    meta2 = Meta(nsub2, NSUP, [NSH] * NC, _cells_of(e2_data, nsub2, nch2))

    # ---------------- e3 from local w-major review table ------------------
    NSUP3 = 8
    nsub3 = -(-MSH // 128)
    nsub3 = -(-nsub3 // NSUP3) * NSUP3
    rows_rev = nsub2 * 128
    wpc3 = []
    w = nsub2
    while w > 0:
        wpc3.append(min(256, w))
        w -= 256
    nch3 = len(wpc3)
    chunk_rows3 = [wp * 128 for wp in wpc3]
    bounds3 = np.cumsum([0] + wpc3) * 128
    e3_pc = []
    for d, s in e3_data:
        c = np.searchsorted(bounds3, s, side="right") - 1
        e3_pc.append((d, c, s - bounds3[c]))
    meta3 = Meta(nsub3, NSUP3, chunk_rows3, _cells_of(e3_pc, nsub3, nch3))

    # ---------------- per-core input arrays -------------------------------
    # emb in w-major layouts
    embA = np.zeros((NC, nsub1, 128, D), np.float32)
    for i in range(NC):
        loc = x0[i * NSH:(i + 1) * NSH]
        r = np.arange(NSH)
        embA[i, r // 128, r % 128] = loc
    emb_wm_parts = []
    cb = np.cumsum([0] + wpc1)
    for c in range(nch1):
        emb_wm_parts.append(
            embA[:, cb[c]:cb[c + 1]].reshape(-1, D))
    emb_wm = np.ascontiguousarray(np.concatenate(emb_wm_parts, 0))

    in_maps = []
    for i in range(NC):
        d1, c1, ix1 = e1_pc[i]
        idx1, dexp1 = meta1.pack(d1, c1, ix1)
        cnt1 = np.bincount(d1, minlength=nsub1 * 128)
        inv1 = (1.0 / np.maximum(cnt1, 1)).reshape(nsub1, 128).T.astype(np.float32)
        d2, c2, ix2 = e2_data[i]
        idx2, dexp2 = meta2.pack(d2, c2, ix2)
        inv2 = np.zeros((128, nsub2), np.float32)
        li = np.arange(len(cons_lists[i]))
        inv2[li % 128, li // 128] = inv2_list[i]
        d3, c3, ix3 = e3_pc[i]
        idx3, dexp3 = meta3.pack(d3, c3, ix3)
        in_maps.append({
            "emb_wm": emb_wm,
            "emb_local": np.ascontiguousarray(embA[i]),
            "idx_e1": idx1, "dx_e1": dexp1, "inv1": np.ascontiguousarray(inv1),
            "idx_e2": idx2, "dx_e2": dexp2, "inv2": inv2,
            "idx_e3": idx3, "dx_e3": dexp3,
            "iota": np.tile(np.arange(128, dtype=np.float32), (128, 1)),
            "vrep": np.tile(v, (128, 1)).astype(np.float32),
            "crep": np.full((128, 1), c_sc, np.float32),
        })

    # ---------------- build device program --------------------------------
    nc = bacc.Bacc("TRN2", target_bir_lowering=False, debug=False,
                   num_devices=NC, num_swdge_queues=CFG["NQ"])

    def din(name, arr):
        return nc.dram_tensor(name, list(arr.shape),
                              mybir.dt.from_np(arr.dtype), kind="ExternalInput")

    t = {k: din(k, in_maps[0][k]) for k in in_maps[0]}
    out_t = nc.dram_tensor("out", [nsub3, 128, D], f32, kind="ExternalOutput")

    qstate = [0]
    rg = [list(range(NC))]

    with tile.TileContext(nc) as tc:
        with (
            tc.tile_pool(name="psum", bufs=2, space="PSUM") as psum_p,
            tc.tile_pool(name="psum2", bufs=2, space="PSUM") as psum2_p,
            tc.tile_pool(name="gather", bufs=4) as gather_p,
            tc.tile_pool(name="gatherb", bufs=4) as gatherb_p,
            tc.tile_pool(name="idx", bufs=6) as idx_p,
            tc.tile_pool(name="dloc", bufs=6) as dloc_p,
            tc.tile_pool(name="oh", bufs=3) as oh_p,
            tc.tile_pool(name="stacc", bufs=1) as stacc_p,
            tc.tile_pool(name="stage", bufs=2) as stage_p,
            tc.tile_pool(name="ic", bufs=3) as ic_p,
            tc.tile_pool(name="tmpb", bufs=3) as tmpb_p,
            tc.tile_pool(name="zeb", bufs=3) as zeb_p,
            tc.tile_pool(name="ze", bufs=3) as ze_p,
            tc.tile_pool(name="den", bufs=4) as den_p,
            tc.tile_pool(name="const", bufs=1) as const_p,
            tc.tile_pool(name="ro", bufs=2) as ro_p,
            tc.tile_pool(name="dram", bufs=1, space="DRAM") as dram_p,
        ):
            pools = {"psum": psum_p, "psum2": psum2_p, "gather": gather_p,
                     "gatherb": gatherb_p, "idx": idx_p, "dloc": dloc_p,
                     "oh": oh_p, "stage": stage_p, "ic": ic_p, "tmpb": tmpb_p,
                     "zeb": zeb_p, "ze": ze_p, "den": den_p, "ro": ro_p}
            iota_t = const_p.tile([128, 128], f32, tag="iota")
            nc.sync.dma_start(out=iota_t[:], in_=t["iota"][:])
            vrep_t = const_p.tile([128, D], f32, tag="vrep")
            nc.sync.dma_start(out=vrep_t[:], in_=t["vrep"][:])
            crep_t = const_p.tile([128, 1], f32, tag="crep")
            nc.sync.dma_start(out=crep_t[:], in_=t["crep"][:])
            t["iota_t"] = iota_t

            x_loc = [dram_p.tile([nsub1, 128, D], f32, tag="x_loc",
                                 name=f"x_loc{l}") for l in range(L)]
            agp = [[dram_p.tile([NC, wpc1[c], 128, D], f32, tag="agp",
                                name=f"agp{l}_{c}", addr_space="Shared")
                    for c in range(nch1)] for l in range(L - 1)]
            xbar_loc = dram_p.tile([nsub1, 128, D], f32, tag="xbar_loc",
                                   name="xbar_loc")
            agx = dram_p.tile([NC, nsub1, 128, D], f32, tag="agx",
                              name="agx", addr_space="Shared")
            rev_loc = dram_p.tile([nsub2, 128, D], f32, tag="rev_loc",
                                  name="rev_loc")
            stacc = [stacc_p.tile([128, NSUP, D], f32, tag=f"stacc{s}",
                                  name=f"stacc{s}")
                     for s in range(meta1.nsuper)]

            cbs = np.cumsum([0] + wpc1)
            pairs = [tuple(x for x in (2 * k, 2 * k + 1) if x < meta1.nsuper)
                     for k in range(nch1)]
            early = list(range(nch1 - 2))
            late = [nch1 - 2, nch1 - 1]
            # ---- propagation layers (pair-ordered, last chunks deferred) ----
            for l in range(L):
                srcv = {}
                for c in range(nch1):
                    if l == 0:
                        base = NC * 128 * int(cbs[c])
                        srcv[c] = t["emb_wm"][base:base + chunk_rows1[c]]
                    else:
                        srcv[c] = agp[l - 1][c][:].rearrange(
                            "i w p d -> (i w p) d")
                for k, pair in enumerate(pairs):
                    for s in pair:
                        _e1_chunks(nc, mybir, pools, meta1, stacc[s], s, srcv,
                                   t, qstate, early, True, NSUP, D)
                    for s in pair:
                        _e1_chunks(nc, mybir, pools, meta1, stacc[s], s, srcv,
                                   t, qstate, late, False, NSUP, D)
                        _e1_norm(nc, mybir, pools, stacc[s], s, t, x_loc[l],
                                 NSUP, D)
                        if l == L - 1:
                            _readout_super(nc, mybir, pools, s, t, x_loc,
                                           xbar_loc, NSUP, D, L)
                    if l < L - 1:
                        nc.gpsimd.collective_compute(
                            "AllGather", mybir.AluOpType.bypass,
                            replica_groups=rg,
                            ins=[x_loc[l][int(cbs[k]):int(cbs[k + 1])]],
                            outs=[agp[l][k][:]])
                    elif k == nch1 - 1:
                        nc.gpsimd.collective_compute(
                            "AllGather", mybir.AluOpType.bypass,
                            replica_groups=rg,
                            ins=[xbar_loc[:]],
                            outs=[agx[:]])

            # ---- e2 ----
            for s in range(meta2.nsuper):
                srcv = {c: agx[c].rearrange("w p d -> (w p) d")
                        for c in range(NC)}
                _e2_super(nc, mybir, pools, meta2, s, srcv, t, qstate,
                          rev_loc, NSUP, D, NC)

            # ---- e3 ----
            bounds3c = np.cumsum([0] + chunk_rows3)
            for s in range(meta3.nsuper):
                srcv = {c: rev_loc[:].rearrange("w p d -> (w p) d")[
                            int(bounds3c[c]):int(bounds3c[c + 1])]
                        for c in range(nch3)}
                _e3_super(nc, mybir, pools, meta3, s, srcv, t, qstate,
                          out_t, vrep_t, crep_t, iota_t, NSUP3, D, nch3)

    nc.compile()

    res = run_bass_kernel_spmd(
        nc, in_maps, core_ids=list(range(NC)),
        trace=CFG["TRACE"] or os.environ.get("GNN_TRACE") == "1")
    _LAST["exec_ns"] = res.exec_time_ns
    _LAST["profile_json"] = res.profile_json
    _LAST["results"] = res.results

    out = np.empty((M, D), np.float32)
    for i in range(NC):
        o = res.results[i]["out"]          # [nsub3, 128, D] w-major
        lr = np.arange(MSH)
        out[i * MSH:(i + 1) * MSH] = o[lr // 128, lr % 128]
    return out
